# revision 29
# baseline (speedup 1.0000x reference)
"""Tensor-parallel attention kernel for Trainium2 (8 NeuronCores).

Problem: B=1, L=2048, D=4096, H=32 q-heads, KV=8 kv-heads, HD=128,
partial rotary ROT=64, causal additive mask, o-projection.

Sharding: TP-8 over heads. Core c owns q-heads 4c..4c+3 and kv-head c
(column shard of w_qkv), plus the matching row shard of w_o. Each core
computes a full [L, D] partial of the output; the host sums the 8
partials (the cross-core reduction of the row-sharded o-projection).

Precision plan: fp16 operands everywhere (PSUM accumulation fp32) —
~216 ns per N=512 matmul vs ~272 ns for fp32r, and half the HBM
traffic. (fp8 was simulated end-to-end and rejected: every placement
of e4m3 operands exceeds the 2e-2 rel-err budget — 2.5e-2..5.6e-2 —
and softmax probabilities overflow e4m3's +-240 range.) The exp is
shifted (exp(s-5)); the shift cancels in the normalization.

On-chip orientation: everything transposed so matmuls contract over
the partition dim with no activation transposes:
  qkvT[col, L] = w_qkv.T @ x.T          (w stationary, xT streamed)
  rope:  qT' = qT * cosE + shift32(qT) * sinE'
         (shift32 = swap of partition blocks 0:32/32:64 done by two
          SBUF->SBUF DMAs; the rotate-half sign is folded into sinE')
  ST[k, q]   = kT_tile.T @ qT            (one matmul per k-tile, K=HD=128)
  PT         = exp(ST - 5)  (diagonal tiles then get a 0/1 visibility mul)
  den[*, q]  = ones.T @ PT               (ones-matmul, accumulated over k)
  oT[d, q]   = V_tile.T @ PT             (V from a one-time PE transpose of vT)
  out[l, e]  = (oT/den).T @ w_o_shard    (partial; summed across cores on host)

Schedule (from trace analysis of the 413 us baseline):
  - lb0 qkv runs in two passes (ct 0-2, then ct 3-5) so the first
    block's weight-slab DMA demand is halved while x streams in; the
    3 DMA queues (sync/scalar HWDGE + gpsimd SWDGE) are balanced to
    ~<=1 MB per queue per cg.
  - PE warmup matmuls read a gpsimd-memset tile, so they start at
    ~6 us (vs waiting for a DMA) and the HAM clock-gate is at 8/8
    before the first real matmul.
  - attention and the o-projection are FUSED: o-proj chunks of q-block
    jq-1 are emitted between the attention heads of block jq, so the
    tensor engine always has ready matmuls to fill exp/normalize
    latency (den/ot accumulators are single-buffered to make the PSUM
    budget fit: ST 2x2 + den 1 + ot 1 + oproj 2 = 8 banks).
"""

import sys

for _p in ("/opt/trn_rl_repo", "/root/.axon_site/_ro/trn_rl_repo"):
    if _p not in sys.path:
        sys.path.append(_p)

import numpy as np

B, L, D = 1, 2048, 4096
H, KV, HD = 32, 8, 128
ROT = 64
SCALE = HD ** -0.5
NEG = -1e9
NCORES = 8
HPC = H // NCORES          # q-heads per core (4)
CPC = HPC * HD + 2 * HD    # w_qkv columns per core (768)
NDT = D // 128             # contraction tiles over D (32)
NKT = L // 128             # k tiles (16)
NJQ = L // 512             # 512-wide q blocks (4)
XBLK = 512                 # L-block width in the qkv phase
NLB = L // XBLK            # 4
EXPSHIFT = -5.0            # softmax exp shift; cancels in normalization
MASKNEG = -30000.0         # additive mask for diagonal tiles (fp16-safe)

_cache = {}


def _build(causal: bool):
    import concourse.mybir as mybir
    import concourse.tile as tile
    from concourse import bacc

    F32 = mybir.dt.float32
    F16 = mybir.dt.float16
    EXP = mybir.ActivationFunctionType.Exp

    nc = bacc.Bacc("TRN2", target_bir_lowering=False, debug=False)

    xt = nc.dram_tensor("xt", [NLB, 128, NDT, XBLK], F16, kind="ExternalInput").ap()
    # ct-major: [6, 128, NDT, 128] so lb0's pass A only gates on 3 slabs
    wqkv = nc.dram_tensor("wqkv", [6, 128, NDT, 128], F16, kind="ExternalInput").ap()
    # eg-major: [128, 2, HPC, 2048] so the first o-proj chunks only need
    # the first 2 MB half
    wo = nc.dram_tensor("wo", [128, 2, HPC, 2048], F16, kind="ExternalInput").ap()
    cos_e = nc.dram_tensor("cos_e", [NLB, 128, 2, XBLK], F16, kind="ExternalInput").ap()
    sin_e = nc.dram_tensor("sin_e", [NLB, 128, 2, XBLK], F16, kind="ExternalInput").ap()
    consts = nc.dram_tensor("consts", [128, 128], F16, kind="ExternalInput").ap()
    if causal:
        # 0/1 lower-triangle for the diagonal 128x128 blocks (the mask is
        # trivially 1 outside the block itself)
        dmadd = nc.dram_tensor("dmadd", [128, 128], F16,
                               kind="ExternalInput").ap()
    else:
        mask_t = nc.dram_tensor("mask_t", [L, L], F16, kind="ExternalInput").ap()
    out_p = nc.dram_tensor("out_p", [L, D], F16, kind="ExternalOutput").ap()

    with tile.TileContext(nc) as tc:
        with tc.tile_pool(name="persist", bufs=1) as persist:
            kt_sb = persist.tile([128, L], F16, tag="kt")
            v_sb = persist.tile([128, NKT, 128], F16, tag="v")
            qt_sb = persist.tile([128, HPC, L], F16, tag="qt")
            otn_sb = persist.tile([128, HPC, L], F16, tag="otn")
            ones = persist.tile([128, 128], F16, tag="ones")
            dm_sb = persist.tile([128, 128], F16, tag="dm")
            expb = persist.tile([128, 1], F32, tag="expb")
            warm = persist.tile([128, 384], F16, tag="warm")
            nc.gpsimd.memset(expb, EXPSHIFT)
            nc.gpsimd.memset(warm, 0.25)

            # ---------------- Phase 1: qkv projection + rope ----------------
            with tc.tile_pool(name="wq", bufs=1) as wqp, \
                 tc.tile_pool(name="xb", bufs=2) as xbp, \
                 tc.tile_pool(name="tabs", bufs=2) as tabs, \
                 tc.tile_pool(name="stage", bufs=3) as stage, \
                 tc.tile_pool(name="rotp", bufs=4) as rotp, \
                 tc.tile_pool(name="vtmp", bufs=2) as vtmp, \
                 tc.tile_pool(name="ps1", bufs=6, space="PSUM") as ps1, \
                 tc.tile_pool(name="psw", bufs=2, space="PSUM") as psw:
                wq_sb = wqp.tile([128, 6, NDT, 128], F16)

                # PE warm-up on a memset tile: starts ~6us in (no DMA dep)
                # so the HAM clock gate reaches 8/8 before the real matmuls
                for w_i in range(16):
                    wps = psw.tile([128, 384], F32, tag="warm",
                                   name=f"warm_{w_i}")
                    nc.tensor.matmul(out=wps, lhsT=warm[:, 0:128], rhs=warm,
                                     start=True, stop=True)

                # deferred tail-work (rope DVE / v transposes) per (lb, ct),
                # emitted one-to-two matmul-groups later so the PE never
                # stalls waiting on the ACT copy of a group's PSUM.
                pending = []

                def flush_pending(n=99):
                    while pending and n > 0:
                        pending.pop(0)()
                        n -= 1

                def post_group(lb, ct, acc, cosb, sinb):
                    # last block alternates drains across DVE/ACT so neither
                    # queue backlogs into the attention phase's start
                    last_lb = lb == NLB - 1
                    on_dve = last_lb and ct % 2 == 0
                    if ct == 5:
                        vt = vtmp.tile([128, XBLK], F16, tag="vt",
                                       name=f"vt_{lb}")
                        if on_dve:
                            nc.vector.tensor_copy(vt, acc)
                        else:
                            nc.scalar.copy(out=vt, in_=acc)

                        def fin_v(lb=lb, vt=vt):
                            # DMA-engine transpose: vT [128d, 512l] ->
                            # v [4x128 l-rows, 128 d], keeping the PE free
                            kk = XBLK // 128
                            nc.sync.dma_start_transpose(
                                out=v_sb[:, kk * lb:kk * (lb + 1), :], in_=vt)

                        pending.append(fin_v)
                        return
                    # rope for q (ct 0..3, scaled tables) and k (ct 4)
                    ti = 0 if ct < 4 else 1
                    s_sb = stage.tile([128, XBLK], F16, tag="s_sb",
                                      name=f"s_sb_{lb}_{ct}", bufs=6)
                    if on_dve:
                        nc.vector.tensor_copy(s_sb, acc)
                    else:
                        nc.scalar.copy(out=s_sb, in_=acc)
                    # rotate-half partition swap via SBUF->SBUF DMA, issued
                    # now so it lands before the deferred DVE work needs it
                    rot = rotp.tile([64, XBLK], F16, tag="rot",
                                    name=f"rot_{lb}_{ct}")
                    reng = (nc.sync, nc.scalar)[ct % 2]
                    reng.dma_start(out=rot[0:32, :], in_=s_sb[32:64, :])
                    reng.dma_start(out=rot[32:64, :], in_=s_sb[0:32, :])

                    def fin_rope(ct=ct, s_sb=s_sb, rot=rot, cosb=cosb,
                                 sinb=sinb, ti=ti, lb=lb):
                        ls = slice(lb * XBLK, (lb + 1) * XBLK)
                        dst = kt_sb[:, ls] if ct == 4 else qt_sb[:, ct, ls]
                        # last block's ropes split across GpSimd/DVE so the
                        # DVE queue is clear for jq0's softmax epilogue and
                        # GpSimd still reaches the w_o DMA issue promptly
                        eng = (nc.gpsimd if lb == NLB - 1 and ct % 2 == 0
                               else nc.vector)
                        eng.tensor_mul(dst, s_sb, cosb[:, ti, :])
                        m2 = stage.tile([64, XBLK], F16, tag="m2",
                                        name=f"m2_{lb}_{ct}")
                        eng.tensor_mul(m2, rot, sinb[0:64, ti, :])
                        eng.tensor_add(dst[0:64, :], dst[0:64, :], m2)

                    pending.append(fin_rope)

                for lb in range(NLB):
                    xblk = xbp.tile([128, NDT, XBLK], F16, tag="xblk")
                    cosb = tabs.tile([128, 2, XBLK], F16, tag="cosb")
                    sinb = tabs.tile([128, 2, XBLK], F16, tag="sinb")
                    if lb == 0:
                        # two-pass first block: pass A (ct 0-2) only needs
                        # half the weight slab while x streams in; queue
                        # plan per cg: gpsimd [x-half, ct2], scalar
                        # [x-half, ct5], sync [ct0, ct1]; ct3/ct4 follow
                        # on sync/gpsimd during pass A's compute.
                        for cg in range(4):
                            cgs = slice(cg * 8, cg * 8 + 8)
                            if cg == 0:
                                # quarter-granularity so the very first
                                # matmuls start as early as possible; sync
                                # is dedicated to w so ct0/ct1 land in step
                                # with the x quarters
                                nc.gpsimd.dma_start(out=xblk[:, 0:2, :],
                                                    in_=xt[lb, :, 0:2, :])
                                nc.scalar.dma_start(out=xblk[:, 2:4, :],
                                                    in_=xt[lb, :, 2:4, :])
                                nc.gpsimd.dma_start(out=xblk[:, 4:6, :],
                                                    in_=xt[lb, :, 4:6, :])
                                nc.scalar.dma_start(out=xblk[:, 6:8, :],
                                                    in_=xt[lb, :, 6:8, :])
                                nc.sync.dma_start(out=wq_sb[:, 0, 0:4, :],
                                                  in_=wqkv[0, :, 0:4, :])
                                nc.sync.dma_start(out=wq_sb[:, 0, 4:8, :],
                                                  in_=wqkv[0, :, 4:8, :])
                                nc.sync.dma_start(out=wq_sb[:, 1, 0:4, :],
                                                  in_=wqkv[1, :, 0:4, :])
                                nc.sync.dma_start(out=wq_sb[:, 1, 4:8, :],
                                                  in_=wqkv[1, :, 4:8, :])
                                nc.gpsimd.dma_start(out=wq_sb[:, 2, cgs, :],
                                                    in_=wqkv[2, :, cgs, :])
                            else:
                                nc.gpsimd.dma_start(
                                    out=xblk[:, cg * 8:cg * 8 + 4, :],
                                    in_=xt[lb, :, cg * 8:cg * 8 + 4, :])
                                nc.scalar.dma_start(
                                    out=xblk[:, cg * 8 + 4:cg * 8 + 8, :],
                                    in_=xt[lb, :, cg * 8 + 4:cg * 8 + 8, :])
                                nc.sync.dma_start(out=wq_sb[:, 0, cgs, :],
                                                  in_=wqkv[0, :, cgs, :])
                                nc.sync.dma_start(out=wq_sb[:, 1, cgs, :],
                                                  in_=wqkv[1, :, cgs, :])
                                nc.gpsimd.dma_start(out=wq_sb[:, 2, cgs, :],
                                                    in_=wqkv[2, :, cgs, :])
                        # pass-B slabs + ct5 behind all of pass A's traffic
                        for cg in range(4):
                            cgs = slice(cg * 8, cg * 8 + 8)
                            nc.sync.dma_start(out=wq_sb[:, 3, cgs, :],
                                              in_=wqkv[3, :, cgs, :])
                            nc.gpsimd.dma_start(out=wq_sb[:, 4, cgs, :],
                                                in_=wqkv[4, :, cgs, :])
                            nc.scalar.dma_start(out=wq_sb[:, 5, cgs, :],
                                                in_=wqkv[5, :, cgs, :])
                        # rope tables aren't read until the first rope finish
                        # (~25us in); keep them behind the critical x chunks
                        nc.scalar.dma_start(out=cosb, in_=cos_e[lb])
                        nc.scalar.dma_start(out=sinb, in_=sin_e[lb])
                        accs0 = {ct: ps1.tile([128, XBLK], F32, tag="acc",
                                              name=f"acc0_{ct}")
                                 for ct in range(6)}
                        for cts in ((0, 1, 2), (3, 4, 5)):
                            for cg in range(4):
                                for ct in cts:
                                    for dti in range(cg * 8, cg * 8 + 8):
                                        nc.tensor.matmul(
                                            out=accs0[ct],
                                            lhsT=wq_sb[:, ct, dti, :],
                                            rhs=xblk[:, dti, :],
                                            start=(dti == 0),
                                            stop=(dti == NDT - 1))
                            for ct in cts:
                                post_group(lb, ct, accs0[ct], cosb, sinb)
                        continue
                    # chunked so block-1 matmuls can start before the whole
                    # 4 MB block has landed (blocks 2-3 are prefetched anyway);
                    # block 1 splits across two queues since it races block-0
                    # traffic
                    for cg in range(4):
                        xeng = nc.scalar if (lb == 1 and cg % 2 == 1) else nc.gpsimd
                        xeng.dma_start(out=xblk[:, cg * 8:cg * 8 + 8, :],
                                       in_=xt[lb, :, cg * 8:cg * 8 + 8, :])
                    nc.sync.dma_start(out=cosb, in_=cos_e[lb])
                    nc.sync.dma_start(out=sinb, in_=sin_e[lb])
                    if lb == 1 and causal:
                        # needed from phase 2 on; off the hot queues
                        nc.scalar.dma_start(out=dm_sb, in_=dmadd)
                    for ct in range(6):
                        acc = ps1.tile([128, XBLK], F32, tag="acc",
                                       name=f"acc_{lb}_{ct}")
                        for dti in range(NDT):
                            nc.tensor.matmul(
                                out=acc,
                                lhsT=wq_sb[:, ct, dti, :],
                                rhs=xblk[:, dti, :],
                                start=(dti == 0), stop=(dti == NDT - 1))
                        # drain faster in the last block so the rope tail
                        # doesn't delay the phase-2 PSUM handoff
                        flush_pending(3 if lb == NLB - 1 else 2)
                        post_group(lb, ct, acc, cosb, sinb)
                flush_pending()
                # ones for the den matmuls: first read at jq0, tiny transfer
                nc.sync.dma_start(out=ones, in_=consts)

            # ---------------- Fused phase 2+3: attention + o-proj -----------
            # PSUM budget: ST 2x[128,2,512] (4 banks) + den (1) + ot (1)
            # + o-proj accs 2x (2) = 8 banks.
            with tc.tile_pool(name="wop", bufs=1) as wop, \
                 tc.tile_pool(name="ptp", bufs=6) as ptp, \
                 tc.tile_pool(name="mb", bufs=2) as mbp, \
                 tc.tile_pool(name="rdp", bufs=2) as rdp, \
                 tc.tile_pool(name="ostp", bufs=3) as ostp, \
                 tc.tile_pool(name="ps_st", bufs=2, space="PSUM") as ps_st, \
                 tc.tile_pool(name="ps_den", bufs=1, space="PSUM") as ps_den, \
                 tc.tile_pool(name="ps_ot", bufs=1, space="PSUM") as ps_ot, \
                 tc.tile_pool(name="ps3", bufs=2, space="PSUM") as ps3:
                wo_sb = wop.tile([128, 2, HPC, 2048], F16)
                nc.gpsimd.dma_start(out=wo_sb[:, 0], in_=wo[:, 0])
                nc.sync.dma_start(out=wo_sb[:, 1], in_=wo[:, 1])

                # o-proj chunk queue: chunks for q-block jq become ready when
                # jq's last head normalizes; they're emitted between the
                # attention heads of jq+1 as PE filler work.
                chunks = []          # pending (lt, eg) descriptors

                def emit_chunk(lt, eg, tail=False):
                    lsl = slice(lt * 128, (lt + 1) * 128)
                    ost = ostp.tile([128, 4, 512], F16, tag="ost",
                                    name=f"ost_{lt}_{eg}")
                    for e4 in range(4):
                        acc = ps3.tile([128, 512], F32, tag="acc3",
                                       name=f"acc3_{lt}_{eg}_{e4}")
                        for h in range(HPC):
                            nc.tensor.matmul(
                                out=acc,
                                lhsT=otn_sb[:, h, lsl],
                                rhs=wo_sb[:, eg, h, e4 * 512:(e4 + 1) * 512],
                                start=(h == 0), stop=(h == HPC - 1))
                        # drains on DVE: the ACT queue stays clear for exps.
                        # The kernel-tail chunks split across both engines
                        # and DMA out in halves so the drain isn't serial.
                        if tail and e4 % 2 == 1:
                            nc.scalar.copy(out=ost[:, e4, :], in_=acc)
                        else:
                            nc.vector.tensor_copy(ost[:, e4, :], acc)
                        if tail and e4 == 1:
                            nc.sync.dma_start(
                                out=out_p[lsl, eg * 2048:eg * 2048 + 1024],
                                in_=ost[:, 0:2, :])
                    if tail:
                        nc.scalar.dma_start(
                            out=out_p[lsl, eg * 2048 + 1024:(eg + 1) * 2048],
                            in_=ost[:, 2:4, :])
                    else:
                        nc.sync.dma_start(
                            out=out_p[lsl, eg * 2048:(eg + 1) * 2048], in_=ost)

                def drain_chunks(n, final=False):
                    while chunks and n > 0:
                        emit_chunk(*chunks.pop(0), tail=final and len(chunks) < 2)
                        n -= 1

                for jq in range(NJQ):
                    qs = slice(jq * 512, (jq + 1) * 512)
                    nkt = 4 * (jq + 1) if causal else NKT
                    diag0 = 4 * jq
                    if not causal:
                        mblk = mbp.tile([128, NKT, 512], F16, tag="mblk")
                        nc.sync.dma_start(
                            out=mblk,
                            in_=mask_t[:, qs].rearrange("(kt p) q -> p kt q", p=128))
                    npair = nkt // 2
                    # diagonal pairs first: their DVE mask-muls then
                    # overlap later pairs' matmuls instead of sitting on
                    # the exp critical path at the iteration tail
                    if causal and npair >= 2:
                        order = [npair - 2, npair - 1] + list(range(npair - 2))
                    else:
                        order = list(range(npair))

                    hctx = {}

                    def get_ctx(h, jq=jq):
                        if h not in hctx:
                            # jq0 has no o-proj filler yet, so its odd heads
                            # borrow the (idle) o-proj acc banks to avoid
                            # den/ot bank serialization at head boundaries
                            if jq == 0 and h % 2 == 1:
                                pd = po = ps3
                                td = to = "acc3"
                            else:
                                pd, po, td, to = ps_den, ps_ot, "den", "ot"
                            hctx[h] = {
                                "den": pd.tile([128, 512], F32, tag=td,
                                               name=f"den_{jq}_{h}"),
                                "ot": po.tile([128, 512], F32, tag=to,
                                              name=f"ot_{jq}_{h}"),
                                "n": 0,
                            }
                        return hctx[h]

                    def tile_off(i, diag0=diag0):
                        # visible region of k-tile i is q >= 128*o for
                        # diagonal offset o; off-diagonal tiles are full
                        if not causal or i < diag0:
                            return 0
                        return 128 * (i - diag0)

                    def emit_den_ot(h, g, pts, jq=jq, diag0=diag0,
                                    npair=npair, tile_off=tile_off,
                                    get_ctx=get_ctx):
                        c = get_ctx(h)
                        pt = pts.pop((h, g))
                        if causal:
                            # 0/1 visibility applied to the fp16
                            # probabilities, off the exp critical path;
                            # only the diagonal 128x128 block is nontrivial
                            for t in (0, 1):
                                i = 2 * g + t
                                if i >= diag0:
                                    q0 = 128 * (i - diag0)
                                    nc.vector.tensor_mul(
                                        pt[:, t, q0:q0 + 128],
                                        pt[:, t, q0:q0 + 128], dm_sb)
                        for t in (0, 1):
                            i = 2 * g + t
                            q0 = tile_off(i)
                            first = c["n"] == 0
                            last = c["n"] == 2 * npair - 1
                            # the first emitted matmul is always the
                            # full-width diagonal o=0 tile, so start=True
                            # initializes the whole PSUM row
                            nc.tensor.matmul(
                                out=c["den"][:, q0:], lhsT=ones,
                                rhs=pt[:, t, q0:],
                                start=first, stop=last)
                            nc.tensor.matmul(
                                out=c["ot"][:, q0:], lhsT=v_sb[:, i, :],
                                rhs=pt[:, t, q0:],
                                start=first, stop=last)
                            c["n"] += 1
                        if last:
                            rd = rdp.tile([128, 512], F32, tag="rd",
                                          name=f"rd_{jq}_{h}")
                            nc.vector.reciprocal_approx_fast(
                                out=rd, in_=c["den"])
                            nc.vector.tensor_mul(
                                otn_sb[:, h, qs], c["ot"], rd)
                            # PE filler: o-proj chunks of the previous block
                            drain_chunks(2)

                    pts = {}
                    inflight = []
                    la = 4 if jq == 0 else 3
                    for h in range(HPC):
                        for g in order:
                            st = ps_st.tile([128, 2, 512], F32, tag="st",
                                            name=f"st_{jq}_{h}_{g}")
                            for t in (0, 1):
                                i = 2 * g + t
                                q0 = tile_off(i)
                                nc.tensor.matmul(
                                    out=st[:, t, q0:],
                                    lhsT=kt_sb[:, i * 128:(i + 1) * 128],
                                    rhs=qt_sb[:, h, jq * 512 + q0:(jq + 1) * 512],
                                    start=True, stop=True)
                            if not causal:
                                for t in (0, 1):
                                    nc.vector.tensor_add(
                                        st[:, t, :], st[:, t, :],
                                        mblk[:, 2 * g + t, :])
                            pt = ptp.tile([128, 2, 512], F16, tag="pt",
                                          name=f"pt_{jq}_{h}_{g}")
                            nc.scalar.activation(pt, st, EXP, bias=expb)
                            pts[(h, g)] = pt
                            inflight.append((h, g))
                            if len(inflight) > la:
                                emit_den_ot(*inflight.pop(0), pts)
                    while inflight:
                        emit_den_ot(*inflight.pop(0), pts)
                    # queue this block's o-proj chunks (eg-major so the
                    # first chunks only need the first half of w_o)
                    for eg in range(2):
                        for lt in range(4 * jq, 4 * jq + 4):
                            chunks.append((lt, eg))
                # tail: the last block's chunks
                drain_chunks(99, final=True)

    nc.compile()
    return nc


def _host_inputs(x, attention_mask, cos, sin, w_qkv, w_o, causal):
    """Build the 8 per-core input maps (fp16, partition-major)."""
    F16 = np.float16

    # x.T pre-tiled: [NLB, 128, NDT, XBLK], contiguous per partition
    xT = np.ascontiguousarray(x[0].T)                     # [D, L]
    xt_h = np.ascontiguousarray(
        xT.reshape(NDT, 128, NLB, XBLK).transpose(2, 1, 0, 3)).astype(F16)
    q_pos = H * HD
    kv_pos = q_pos + KV * HD

    # extended rope tables: slot 0 = q (scale folded), slot 1 = k
    # row d<64: cos[l, d]; row d>=64: 1.0 (cos) / 0.0 (sin)
    # rotate-half sign folded into sin: rows 0:32 negated
    cos_t = cos.T.astype(np.float32)                      # [ROT, L]
    sin_t = sin.T.astype(np.float32)
    cos_e = np.empty((2, 128, L), np.float32)
    sin_e = np.zeros((2, 128, L), np.float32)
    cos_e[0, :ROT] = cos_t * SCALE
    cos_e[0, ROT:] = SCALE
    cos_e[1, :ROT] = cos_t
    cos_e[1, ROT:] = 1.0
    sin_e[0, :ROT] = sin_t * SCALE
    sin_e[1, :ROT] = sin_t
    sin_e[:, :32] *= -1.0
    # -> [NLB, 128, 2, XBLK]
    cs_h = np.ascontiguousarray(
        cos_e.reshape(2, 128, NLB, XBLK).transpose(2, 1, 0, 3)).astype(F16)
    sn_h = np.ascontiguousarray(
        sin_e.reshape(2, 128, NLB, XBLK).transpose(2, 1, 0, 3)).astype(F16)

    consts = np.ones((128, 128), np.float32).astype(F16)

    mask2d = np.ascontiguousarray(attention_mask[0, 0])   # [L(q), L(k)]
    if causal:
        mask_t_full = None
        # 0/1 lower-triangle [k_local, q_local] for the diagonal blocks
        kloc = np.arange(128)[:, None]
        qloc = np.arange(128)[None, :]
        dmadd = np.ascontiguousarray((qloc >= kloc).astype(F16))
    else:
        mask_t_full = np.ascontiguousarray(
            np.maximum(mask2d.T, MASKNEG)).astype(F16)    # [k, q]
        dmadd = None

    in_maps = []
    for c in range(NCORES):
        cols = []
        for j in range(HPC):
            h = c * HPC + j
            cols.append(w_qkv[:, h * HD:(h + 1) * HD])
        cols.append(w_qkv[:, q_pos + c * HD:q_pos + (c + 1) * HD])
        cols.append(w_qkv[:, kv_pos + c * HD:kv_pos + (c + 1) * HD])
        wqkv_c = np.concatenate(cols, axis=1)             # [D, 768]
        # ct-major slabs: [6, 128, NDT, 128]
        wqkv_h = np.ascontiguousarray(
            wqkv_c.reshape(NDT, 128, 6, 128).transpose(2, 1, 0, 3)).astype(F16)
        wo_c = w_o[c * HPC * HD:(c + 1) * HPC * HD, :]    # [512, D]
        # eg-major: [128, 2, HPC, 2048]
        wo_h = np.ascontiguousarray(
            wo_c.reshape(HPC, 128, 2, 2048).transpose(1, 2, 0, 3)).astype(F16)
        m = {"xt": xt_h, "wqkv": wqkv_h, "wo": wo_h,
             "cos_e": cs_h, "sin_e": sn_h, "consts": consts}
        if causal:
            m["dmadd"] = dmadd
        else:
            m["mask_t"] = mask_t_full
        in_maps.append(m)
    return in_maps


def _is_causal(mask2d):
    expected = np.where(
        np.tril(np.ones((L, L), dtype=bool)), np.float32(0.0), np.float32(NEG))
    return mask2d.shape == (L, L) and np.array_equal(mask2d, expected)


def kernel(x, attention_mask, cos, sin, w_qkv, w_o, _trace=False):
    from concourse.bass_utils import run_bass_kernel_spmd

    x = np.asarray(x, dtype=np.float32)
    attention_mask = np.asarray(attention_mask, dtype=np.float32)
    cos = np.asarray(cos, dtype=np.float32)
    sin = np.asarray(sin, dtype=np.float32)
    w_qkv = np.asarray(w_qkv, dtype=np.float32)
    w_o = np.asarray(w_o, dtype=np.float32)

    causal = _is_causal(attention_mask[0, 0])
    if causal not in _cache:
        _cache[causal] = _build(causal)
    nc = _cache[causal]

    in_maps = _host_inputs(x, attention_mask, cos, sin, w_qkv, w_o, causal)
    try:
        res = run_bass_kernel_spmd(nc, in_maps, list(range(NCORES)), trace=_trace)
    except Exception:
        # transient device errors (e.g. NRT_EXEC_UNIT_UNRECOVERABLE) usually
        # clear on retry
        res = run_bass_kernel_spmd(nc, in_maps, list(range(NCORES)), trace=_trace)
    out = np.zeros((L, D), np.float64)
    for c in range(NCORES):
        out += res.results[c]["out_p"].astype(np.float64)
    if _trace:
        kernel._last_exec_time_ns = res.exec_time_ns
        kernel._last_res = res
    return out.astype(np.float32).reshape(B, L, D)


# revision 30
# speedup vs baseline: 1.0060x; 1.0060x over previous
"""Tensor-parallel attention kernel for Trainium2 (8 NeuronCores).

Problem: B=1, L=2048, D=4096, H=32 q-heads, KV=8 kv-heads, HD=128,
partial rotary ROT=64, causal additive mask, o-projection.

Sharding: TP-8 over heads. Core c owns q-heads 4c..4c+3 and kv-head c
(column shard of w_qkv), plus the matching row shard of w_o. Each core
computes a full [L, D] partial of the output; the host sums the 8
partials (the cross-core reduction of the row-sharded o-projection).

Precision plan: fp16 operands everywhere (PSUM accumulation fp32) —
~216 ns per N=512 matmul vs ~272 ns for fp32r, and half the HBM
traffic. (fp8 was simulated end-to-end and rejected: every placement
of e4m3 operands exceeds the 2e-2 rel-err budget — 2.5e-2..5.6e-2 —
and softmax probabilities overflow e4m3's +-240 range.) The exp is
shifted (exp(s-5)); the shift cancels in the normalization.

On-chip orientation: everything transposed so matmuls contract over
the partition dim with no activation transposes:
  qkvT[col, L] = w_qkv.T @ x.T          (w stationary, xT streamed)
  rope:  qT' = qT * cosE + shift32(qT) * sinE'
         (shift32 = swap of partition blocks 0:32/32:64 done by two
          SBUF->SBUF DMAs; the rotate-half sign is folded into sinE')
  ST[k, q]   = kT_tile.T @ qT            (one matmul per k-tile, K=HD=128)
  PT         = exp(ST - 5)  (diagonal tiles then get a 0/1 visibility mul)
  den[*, q]  = ones.T @ PT               (ones-matmul, accumulated over k)
  oT[d, q]   = V_tile.T @ PT             (V from a one-time PE transpose of vT)
  out[l, e]  = (oT/den).T @ w_o_shard    (partial; summed across cores on host)

Schedule (from trace analysis of the 413 us baseline):
  - lb0 qkv runs in two passes (ct 0-2, then ct 3-5) so the first
    block's weight-slab DMA demand is halved while x streams in; the
    3 DMA queues (sync/scalar HWDGE + gpsimd SWDGE) are balanced to
    ~<=1 MB per queue per cg.
  - PE warmup matmuls read a gpsimd-memset tile, so they start at
    ~6 us (vs waiting for a DMA) and the HAM clock-gate is at 8/8
    before the first real matmul.
  - attention and the o-projection are FUSED: o-proj chunks of q-block
    jq-1 are emitted between the attention heads of block jq, so the
    tensor engine always has ready matmuls to fill exp/normalize
    latency (den/ot accumulators are single-buffered to make the PSUM
    budget fit: ST 2x2 + den 1 + ot 1 + oproj 2 = 8 banks).
"""

import sys

for _p in ("/opt/trn_rl_repo", "/root/.axon_site/_ro/trn_rl_repo"):
    if _p not in sys.path:
        sys.path.append(_p)

import numpy as np

B, L, D = 1, 2048, 4096
H, KV, HD = 32, 8, 128
ROT = 64
SCALE = HD ** -0.5
NEG = -1e9
NCORES = 8
HPC = H // NCORES          # q-heads per core (4)
CPC = HPC * HD + 2 * HD    # w_qkv columns per core (768)
NDT = D // 128             # contraction tiles over D (32)
NKT = L // 128             # k tiles (16)
NJQ = L // 512             # 512-wide q blocks (4)
XBLK = 512                 # L-block width in the qkv phase
NLB = L // XBLK            # 4
EXPSHIFT = -5.0            # softmax exp shift; cancels in normalization
MASKNEG = -30000.0         # additive mask for diagonal tiles (fp16-safe)

_cache = {}


def _build(causal: bool):
    import concourse.mybir as mybir
    import concourse.tile as tile
    from concourse import bacc

    F32 = mybir.dt.float32
    F16 = mybir.dt.float16
    EXP = mybir.ActivationFunctionType.Exp

    nc = bacc.Bacc("TRN2", target_bir_lowering=False, debug=False)

    xt = nc.dram_tensor("xt", [NLB, 128, NDT, XBLK], F16, kind="ExternalInput").ap()
    # ct-major: [6, 128, NDT, 128] so lb0's pass A only gates on 3 slabs
    wqkv = nc.dram_tensor("wqkv", [6, 128, NDT, 128], F16, kind="ExternalInput").ap()
    # eg-major: [128, 2, HPC, 2048] so the first o-proj chunks only need
    # the first 2 MB half
    wo = nc.dram_tensor("wo", [128, 2, HPC, 2048], F16, kind="ExternalInput").ap()
    cos_e = nc.dram_tensor("cos_e", [NLB, 128, 2, XBLK], F16, kind="ExternalInput").ap()
    sin_e = nc.dram_tensor("sin_e", [NLB, 128, 2, XBLK], F16, kind="ExternalInput").ap()
    consts = nc.dram_tensor("consts", [128, 128], F16, kind="ExternalInput").ap()
    if causal:
        # 0/1 lower-triangle for the diagonal 128x128 blocks (the mask is
        # trivially 1 outside the block itself)
        dmadd = nc.dram_tensor("dmadd", [128, 128], F16,
                               kind="ExternalInput").ap()
    else:
        mask_t = nc.dram_tensor("mask_t", [L, L], F16, kind="ExternalInput").ap()
    out_p = nc.dram_tensor("out_p", [L, D], F16, kind="ExternalOutput").ap()

    with tile.TileContext(nc) as tc:
        with tc.tile_pool(name="persist", bufs=1) as persist:
            kt_sb = persist.tile([128, L], F16, tag="kt")
            v_sb = persist.tile([128, NKT, 128], F16, tag="v")
            qt_sb = persist.tile([128, HPC, L], F16, tag="qt")
            otn_sb = persist.tile([128, HPC, L], F16, tag="otn")
            ones = persist.tile([128, 128], F16, tag="ones")
            dm_sb = persist.tile([128, 128], F16, tag="dm")
            expb = persist.tile([128, 1], F32, tag="expb")
            warm = persist.tile([128, 384], F16, tag="warm")
            nc.gpsimd.memset(expb, EXPSHIFT)
            nc.gpsimd.memset(warm, 0.25)

            # ---------------- Phase 1: qkv projection + rope ----------------
            with tc.tile_pool(name="wq", bufs=1) as wqp, \
                 tc.tile_pool(name="xb", bufs=2) as xbp, \
                 tc.tile_pool(name="tabs", bufs=2) as tabs, \
                 tc.tile_pool(name="stage", bufs=3) as stage, \
                 tc.tile_pool(name="rotp", bufs=4) as rotp, \
                 tc.tile_pool(name="vtmp", bufs=2) as vtmp, \
                 tc.tile_pool(name="ps1", bufs=6, space="PSUM") as ps1, \
                 tc.tile_pool(name="psw", bufs=2, space="PSUM") as psw:
                wq_sb = wqp.tile([128, 6, NDT, 128], F16)

                # PE warm-up on a memset tile: starts ~6us in (no DMA dep)
                # so the HAM clock gate reaches 8/8 before the real matmuls
                for w_i in range(16):
                    wps = psw.tile([128, 384], F32, tag="warm",
                                   name=f"warm_{w_i}")
                    nc.tensor.matmul(out=wps, lhsT=warm[:, 0:128], rhs=warm,
                                     start=True, stop=True)

                # deferred tail-work (rope DVE / v transposes) per (lb, ct),
                # emitted one-to-two matmul-groups later so the PE never
                # stalls waiting on the ACT copy of a group's PSUM.
                pending = []

                def flush_pending(n=99):
                    while pending and n > 0:
                        pending.pop(0)()
                        n -= 1

                def post_group(lb, ct, acc, cosb, sinb):
                    # last block alternates drains across DVE/ACT so neither
                    # queue backlogs into the attention phase's start
                    last_lb = lb == NLB - 1
                    on_dve = last_lb and ct % 2 == 0
                    if ct == 5:
                        vt = vtmp.tile([128, XBLK], F16, tag="vt",
                                       name=f"vt_{lb}")
                        if on_dve:
                            nc.vector.tensor_copy(vt, acc)
                        else:
                            nc.scalar.copy(out=vt, in_=acc)

                        def fin_v(lb=lb, vt=vt):
                            # DMA-engine transpose: vT [128d, 512l] ->
                            # v [4x128 l-rows, 128 d], keeping the PE free
                            kk = XBLK // 128
                            nc.sync.dma_start_transpose(
                                out=v_sb[:, kk * lb:kk * (lb + 1), :], in_=vt)

                        pending.append(fin_v)
                        return
                    # rope for q (ct 0..3, scaled tables) and k (ct 4)
                    ti = 0 if ct < 4 else 1
                    s_sb = stage.tile([128, XBLK], F16, tag="s_sb",
                                      name=f"s_sb_{lb}_{ct}", bufs=6)
                    if on_dve:
                        nc.vector.tensor_copy(s_sb, acc)
                    else:
                        nc.scalar.copy(out=s_sb, in_=acc)
                    # rotate-half partition swap via SBUF->SBUF DMA, issued
                    # now so it lands before the deferred DVE work needs it
                    rot = rotp.tile([64, XBLK], F16, tag="rot",
                                    name=f"rot_{lb}_{ct}")
                    reng = (nc.sync, nc.scalar)[ct % 2]
                    reng.dma_start(out=rot[0:32, :], in_=s_sb[32:64, :])
                    reng.dma_start(out=rot[32:64, :], in_=s_sb[0:32, :])

                    def fin_rope(ct=ct, s_sb=s_sb, rot=rot, cosb=cosb,
                                 sinb=sinb, ti=ti, lb=lb):
                        ls = slice(lb * XBLK, (lb + 1) * XBLK)
                        dst = kt_sb[:, ls] if ct == 4 else qt_sb[:, ct, ls]
                        # last block's ropes split across GpSimd/DVE so the
                        # DVE queue is clear for jq0's softmax epilogue and
                        # GpSimd still reaches the w_o DMA issue promptly
                        eng = (nc.gpsimd if lb == NLB - 1 and ct % 2 == 0
                               else nc.vector)
                        eng.tensor_mul(dst, s_sb, cosb[:, ti, :])
                        m2 = stage.tile([64, XBLK], F16, tag="m2",
                                        name=f"m2_{lb}_{ct}")
                        eng.tensor_mul(m2, rot, sinb[0:64, ti, :])
                        eng.tensor_add(dst[0:64, :], dst[0:64, :], m2)

                    pending.append(fin_rope)

                for lb in range(NLB):
                    xblk = xbp.tile([128, NDT, XBLK], F16, tag="xblk")
                    cosb = tabs.tile([128, 2, XBLK], F16, tag="cosb")
                    sinb = tabs.tile([128, 2, XBLK], F16, tag="sinb")
                    if lb == 0:
                        # two-pass first block: pass A (ct 0-2) only needs
                        # half the weight slab while x streams in; queue
                        # plan per cg: gpsimd [x-half, ct2], scalar
                        # [x-half, ct5], sync [ct0, ct1]; ct3/ct4 follow
                        # on sync/gpsimd during pass A's compute.
                        for cg in range(4):
                            cgs = slice(cg * 8, cg * 8 + 8)
                            if cg == 0:
                                # quarter-granularity so the very first
                                # matmuls start as early as possible; sync
                                # is dedicated to w so ct0/ct1 land in step
                                # with the x quarters
                                nc.gpsimd.dma_start(out=xblk[:, 0:2, :],
                                                    in_=xt[lb, :, 0:2, :])
                                nc.scalar.dma_start(out=xblk[:, 2:4, :],
                                                    in_=xt[lb, :, 2:4, :])
                                nc.gpsimd.dma_start(out=xblk[:, 4:6, :],
                                                    in_=xt[lb, :, 4:6, :])
                                nc.scalar.dma_start(out=xblk[:, 6:8, :],
                                                    in_=xt[lb, :, 6:8, :])
                                nc.sync.dma_start(out=wq_sb[:, 0, 0:4, :],
                                                  in_=wqkv[0, :, 0:4, :])
                                nc.sync.dma_start(out=wq_sb[:, 0, 4:8, :],
                                                  in_=wqkv[0, :, 4:8, :])
                                nc.sync.dma_start(out=wq_sb[:, 1, 0:4, :],
                                                  in_=wqkv[1, :, 0:4, :])
                                nc.sync.dma_start(out=wq_sb[:, 1, 4:8, :],
                                                  in_=wqkv[1, :, 4:8, :])
                                nc.gpsimd.dma_start(out=wq_sb[:, 2, cgs, :],
                                                    in_=wqkv[2, :, cgs, :])
                            else:
                                nc.gpsimd.dma_start(
                                    out=xblk[:, cg * 8:cg * 8 + 4, :],
                                    in_=xt[lb, :, cg * 8:cg * 8 + 4, :])
                                nc.scalar.dma_start(
                                    out=xblk[:, cg * 8 + 4:cg * 8 + 8, :],
                                    in_=xt[lb, :, cg * 8 + 4:cg * 8 + 8, :])
                                nc.sync.dma_start(out=wq_sb[:, 0, cgs, :],
                                                  in_=wqkv[0, :, cgs, :])
                                nc.sync.dma_start(out=wq_sb[:, 1, cgs, :],
                                                  in_=wqkv[1, :, cgs, :])
                                nc.gpsimd.dma_start(out=wq_sb[:, 2, cgs, :],
                                                    in_=wqkv[2, :, cgs, :])
                        # pass-B slabs + ct5 behind all of pass A's traffic
                        for cg in range(4):
                            cgs = slice(cg * 8, cg * 8 + 8)
                            nc.sync.dma_start(out=wq_sb[:, 3, cgs, :],
                                              in_=wqkv[3, :, cgs, :])
                            nc.gpsimd.dma_start(out=wq_sb[:, 4, cgs, :],
                                                in_=wqkv[4, :, cgs, :])
                            nc.scalar.dma_start(out=wq_sb[:, 5, cgs, :],
                                                in_=wqkv[5, :, cgs, :])
                        # rope tables aren't read until the first rope finish
                        # (~25us in); keep them behind the critical x chunks
                        nc.scalar.dma_start(out=cosb, in_=cos_e[lb])
                        nc.scalar.dma_start(out=sinb, in_=sin_e[lb])
                        accs0 = {ct: ps1.tile([128, XBLK], F32, tag="acc",
                                              name=f"acc0_{ct}")
                                 for ct in range(6)}
                        for cts in ((0, 1, 2), (3, 4, 5)):
                            for cg in range(4):
                                for ct in cts:
                                    for dti in range(cg * 8, cg * 8 + 8):
                                        nc.tensor.matmul(
                                            out=accs0[ct],
                                            lhsT=wq_sb[:, ct, dti, :],
                                            rhs=xblk[:, dti, :],
                                            start=(dti == 0),
                                            stop=(dti == NDT - 1))
                            for ct in cts:
                                post_group(lb, ct, accs0[ct], cosb, sinb)
                        continue
                    # chunked so block-1 matmuls can start before the whole
                    # 4 MB block has landed (blocks 2-3 are prefetched anyway);
                    # block 1 splits across two queues since it races block-0
                    # traffic
                    for cg in range(4):
                        xeng = nc.scalar if (lb == 1 and cg % 2 == 1) else nc.gpsimd
                        xeng.dma_start(out=xblk[:, cg * 8:cg * 8 + 8, :],
                                       in_=xt[lb, :, cg * 8:cg * 8 + 8, :])
                    nc.sync.dma_start(out=cosb, in_=cos_e[lb])
                    nc.sync.dma_start(out=sinb, in_=sin_e[lb])
                    if lb == 1 and causal:
                        # needed from phase 2 on; off the hot queues
                        nc.scalar.dma_start(out=dm_sb, in_=dmadd)
                    for ct in range(6):
                        acc = ps1.tile([128, XBLK], F32, tag="acc",
                                       name=f"acc_{lb}_{ct}")
                        for dti in range(NDT):
                            nc.tensor.matmul(
                                out=acc,
                                lhsT=wq_sb[:, ct, dti, :],
                                rhs=xblk[:, dti, :],
                                start=(dti == 0), stop=(dti == NDT - 1))
                        # drain faster in the last block so the rope tail
                        # doesn't delay the phase-2 PSUM handoff
                        flush_pending(3 if lb == NLB - 1 else 2)
                        post_group(lb, ct, acc, cosb, sinb)
                flush_pending()
                # ones for the den matmuls: first read at jq0, tiny transfer
                nc.sync.dma_start(out=ones, in_=consts)

            # ---------------- Fused phase 2+3: attention + o-proj -----------
            # PSUM budget: ST 2x[128,2,512] (4 banks) + den (1) + ot (1)
            # + o-proj accs 2x (2) = 8 banks.
            with tc.tile_pool(name="wop", bufs=1) as wop, \
                 tc.tile_pool(name="ptp", bufs=6) as ptp, \
                 tc.tile_pool(name="mb", bufs=2) as mbp, \
                 tc.tile_pool(name="rdp", bufs=2) as rdp, \
                 tc.tile_pool(name="ostp", bufs=3) as ostp, \
                 tc.tile_pool(name="ps_st", bufs=2, space="PSUM") as ps_st, \
                 tc.tile_pool(name="ps_den", bufs=1, space="PSUM") as ps_den, \
                 tc.tile_pool(name="ps_ot", bufs=1, space="PSUM") as ps_ot, \
                 tc.tile_pool(name="ps3", bufs=2, space="PSUM") as ps3:
                wo_sb = wop.tile([128, 2, HPC, 2048], F16)
                nc.gpsimd.dma_start(out=wo_sb[:, 0], in_=wo[:, 0])
                nc.sync.dma_start(out=wo_sb[:, 1], in_=wo[:, 1])

                # o-proj chunk queue: chunks for q-block jq become ready when
                # jq's last head normalizes; they're emitted between the
                # attention heads of jq+1 as PE filler work.
                chunks = []          # pending (lt, eg) descriptors

                def emit_chunk(lt, eg, tail=False):
                    lsl = slice(lt * 128, (lt + 1) * 128)
                    ost = ostp.tile([128, 4, 512], F16, tag="ost",
                                    name=f"ost_{lt}_{eg}")
                    for e4 in range(4):
                        acc = ps3.tile([128, 512], F32, tag="acc3",
                                       name=f"acc3_{lt}_{eg}_{e4}")
                        for h in range(HPC):
                            nc.tensor.matmul(
                                out=acc,
                                lhsT=otn_sb[:, h, lsl],
                                rhs=wo_sb[:, eg, h, e4 * 512:(e4 + 1) * 512],
                                start=(h == 0), stop=(h == HPC - 1))
                        # drains on DVE: the ACT queue stays clear for exps.
                        # The kernel-tail chunks split across both engines
                        # and DMA out in halves so the drain isn't serial.
                        if tail and e4 % 2 == 1:
                            nc.scalar.copy(out=ost[:, e4, :], in_=acc)
                        else:
                            nc.vector.tensor_copy(ost[:, e4, :], acc)
                        if tail and e4 == 1:
                            nc.sync.dma_start(
                                out=out_p[lsl, eg * 2048:eg * 2048 + 1024],
                                in_=ost[:, 0:2, :])
                    if tail:
                        nc.scalar.dma_start(
                            out=out_p[lsl, eg * 2048 + 1024:(eg + 1) * 2048],
                            in_=ost[:, 2:4, :])
                    else:
                        nc.sync.dma_start(
                            out=out_p[lsl, eg * 2048:(eg + 1) * 2048], in_=ost)

                def drain_chunks(n, final=False):
                    while chunks and n > 0:
                        emit_chunk(*chunks.pop(0), tail=final and len(chunks) < 2)
                        n -= 1

                for jq in range(NJQ):
                    qs = slice(jq * 512, (jq + 1) * 512)
                    nkt = 4 * (jq + 1) if causal else NKT
                    diag0 = 4 * jq
                    if not causal:
                        mblk = mbp.tile([128, NKT, 512], F16, tag="mblk")
                        nc.sync.dma_start(
                            out=mblk,
                            in_=mask_t[:, qs].rearrange("(kt p) q -> p kt q", p=128))
                    npair = nkt // 2
                    # diagonal pairs first: their DVE mask-muls then
                    # overlap later pairs' matmuls instead of sitting on
                    # the exp critical path at the iteration tail
                    if causal and npair >= 2:
                        order = [npair - 2, npair - 1] + list(range(npair - 2))
                    else:
                        order = list(range(npair))

                    hctx = {}

                    def get_ctx(h, jq=jq):
                        if h not in hctx:
                            # jq0 has no o-proj filler yet, so its odd heads
                            # borrow the (idle) o-proj acc banks to avoid
                            # den/ot bank serialization at head boundaries
                            if jq == 0 and h % 2 == 1:
                                pd = po = ps3
                                td = to = "acc3"
                            else:
                                pd, po, td, to = ps_den, ps_ot, "den", "ot"
                            hctx[h] = {
                                "den": pd.tile([128, 512], F32, tag=td,
                                               name=f"den_{jq}_{h}"),
                                "ot": po.tile([128, 512], F32, tag=to,
                                              name=f"ot_{jq}_{h}"),
                                "n": 0,
                            }
                        return hctx[h]

                    def tile_off(i, diag0=diag0):
                        # visible region of k-tile i is q >= 128*o for
                        # diagonal offset o; off-diagonal tiles are full
                        if not causal or i < diag0:
                            return 0
                        return 128 * (i - diag0)

                    def emit_den_ot(h, g, pts, jq=jq, diag0=diag0,
                                    npair=npair, tile_off=tile_off,
                                    get_ctx=get_ctx):
                        c = get_ctx(h)
                        pt = pts.pop((h, g))
                        if causal:
                            # 0/1 visibility applied to the fp16
                            # probabilities, off the exp critical path;
                            # only the diagonal 128x128 block is nontrivial
                            for t in (0, 1):
                                i = 2 * g + t
                                if i >= diag0:
                                    q0 = 128 * (i - diag0)
                                    nc.vector.tensor_mul(
                                        pt[:, t, q0:q0 + 128],
                                        pt[:, t, q0:q0 + 128], dm_sb)
                        for t in (0, 1):
                            i = 2 * g + t
                            q0 = tile_off(i)
                            first = c["n"] == 0
                            last = c["n"] == 2 * npair - 1
                            # the first emitted matmul is always the
                            # full-width diagonal o=0 tile, so start=True
                            # initializes the whole PSUM row
                            nc.tensor.matmul(
                                out=c["den"][:, q0:], lhsT=ones,
                                rhs=pt[:, t, q0:],
                                start=first, stop=last)
                            nc.tensor.matmul(
                                out=c["ot"][:, q0:], lhsT=v_sb[:, i, :],
                                rhs=pt[:, t, q0:],
                                start=first, stop=last)
                            c["n"] += 1
                        if last:
                            rd = rdp.tile([128, 512], F32, tag="rd",
                                          name=f"rd_{jq}_{h}")
                            nc.vector.reciprocal_approx_fast(
                                out=rd, in_=c["den"])
                            nc.vector.tensor_mul(
                                otn_sb[:, h, qs], c["ot"], rd)
                            # PE filler: o-proj chunks of the previous block
                            drain_chunks(2)

                    pts = {}
                    inflight = []
                    la = 4 if jq == 0 else 2
                    for h in range(HPC):
                        for g in order:
                            st = ps_st.tile([128, 2, 512], F32, tag="st",
                                            name=f"st_{jq}_{h}_{g}")
                            for t in (0, 1):
                                i = 2 * g + t
                                q0 = tile_off(i)
                                nc.tensor.matmul(
                                    out=st[:, t, q0:],
                                    lhsT=kt_sb[:, i * 128:(i + 1) * 128],
                                    rhs=qt_sb[:, h, jq * 512 + q0:(jq + 1) * 512],
                                    start=True, stop=True)
                            if not causal:
                                for t in (0, 1):
                                    nc.vector.tensor_add(
                                        st[:, t, :], st[:, t, :],
                                        mblk[:, 2 * g + t, :])
                            pt = ptp.tile([128, 2, 512], F16, tag="pt",
                                          name=f"pt_{jq}_{h}_{g}")
                            nc.scalar.activation(pt, st, EXP, bias=expb)
                            pts[(h, g)] = pt
                            inflight.append((h, g))
                            if len(inflight) > la:
                                emit_den_ot(*inflight.pop(0), pts)
                    while inflight:
                        emit_den_ot(*inflight.pop(0), pts)
                    # queue this block's o-proj chunks (eg-major so the
                    # first chunks only need the first half of w_o)
                    for eg in range(2):
                        for lt in range(4 * jq, 4 * jq + 4):
                            chunks.append((lt, eg))
                # tail: the last block's chunks
                drain_chunks(99, final=True)

    nc.compile()
    return nc


def _host_inputs(x, attention_mask, cos, sin, w_qkv, w_o, causal):
    """Build the 8 per-core input maps (fp16, partition-major)."""
    F16 = np.float16

    # x.T pre-tiled: [NLB, 128, NDT, XBLK], contiguous per partition
    xT = np.ascontiguousarray(x[0].T)                     # [D, L]
    xt_h = np.ascontiguousarray(
        xT.reshape(NDT, 128, NLB, XBLK).transpose(2, 1, 0, 3)).astype(F16)
    q_pos = H * HD
    kv_pos = q_pos + KV * HD

    # extended rope tables: slot 0 = q (scale folded), slot 1 = k
    # row d<64: cos[l, d]; row d>=64: 1.0 (cos) / 0.0 (sin)
    # rotate-half sign folded into sin: rows 0:32 negated
    cos_t = cos.T.astype(np.float32)                      # [ROT, L]
    sin_t = sin.T.astype(np.float32)
    cos_e = np.empty((2, 128, L), np.float32)
    sin_e = np.zeros((2, 128, L), np.float32)
    cos_e[0, :ROT] = cos_t * SCALE
    cos_e[0, ROT:] = SCALE
    cos_e[1, :ROT] = cos_t
    cos_e[1, ROT:] = 1.0
    sin_e[0, :ROT] = sin_t * SCALE
    sin_e[1, :ROT] = sin_t
    sin_e[:, :32] *= -1.0
    # -> [NLB, 128, 2, XBLK]
    cs_h = np.ascontiguousarray(
        cos_e.reshape(2, 128, NLB, XBLK).transpose(2, 1, 0, 3)).astype(F16)
    sn_h = np.ascontiguousarray(
        sin_e.reshape(2, 128, NLB, XBLK).transpose(2, 1, 0, 3)).astype(F16)

    consts = np.ones((128, 128), np.float32).astype(F16)

    mask2d = np.ascontiguousarray(attention_mask[0, 0])   # [L(q), L(k)]
    if causal:
        mask_t_full = None
        # 0/1 lower-triangle [k_local, q_local] for the diagonal blocks
        kloc = np.arange(128)[:, None]
        qloc = np.arange(128)[None, :]
        dmadd = np.ascontiguousarray((qloc >= kloc).astype(F16))
    else:
        mask_t_full = np.ascontiguousarray(
            np.maximum(mask2d.T, MASKNEG)).astype(F16)    # [k, q]
        dmadd = None

    in_maps = []
    for c in range(NCORES):
        cols = []
        for j in range(HPC):
            h = c * HPC + j
            cols.append(w_qkv[:, h * HD:(h + 1) * HD])
        cols.append(w_qkv[:, q_pos + c * HD:q_pos + (c + 1) * HD])
        cols.append(w_qkv[:, kv_pos + c * HD:kv_pos + (c + 1) * HD])
        wqkv_c = np.concatenate(cols, axis=1)             # [D, 768]
        # ct-major slabs: [6, 128, NDT, 128]
        wqkv_h = np.ascontiguousarray(
            wqkv_c.reshape(NDT, 128, 6, 128).transpose(2, 1, 0, 3)).astype(F16)
        wo_c = w_o[c * HPC * HD:(c + 1) * HPC * HD, :]    # [512, D]
        # eg-major: [128, 2, HPC, 2048]
        wo_h = np.ascontiguousarray(
            wo_c.reshape(HPC, 128, 2, 2048).transpose(1, 2, 0, 3)).astype(F16)
        m = {"xt": xt_h, "wqkv": wqkv_h, "wo": wo_h,
             "cos_e": cs_h, "sin_e": sn_h, "consts": consts}
        if causal:
            m["dmadd"] = dmadd
        else:
            m["mask_t"] = mask_t_full
        in_maps.append(m)
    return in_maps


def _is_causal(mask2d):
    expected = np.where(
        np.tril(np.ones((L, L), dtype=bool)), np.float32(0.0), np.float32(NEG))
    return mask2d.shape == (L, L) and np.array_equal(mask2d, expected)


def kernel(x, attention_mask, cos, sin, w_qkv, w_o, _trace=False):
    from concourse.bass_utils import run_bass_kernel_spmd

    x = np.asarray(x, dtype=np.float32)
    attention_mask = np.asarray(attention_mask, dtype=np.float32)
    cos = np.asarray(cos, dtype=np.float32)
    sin = np.asarray(sin, dtype=np.float32)
    w_qkv = np.asarray(w_qkv, dtype=np.float32)
    w_o = np.asarray(w_o, dtype=np.float32)

    causal = _is_causal(attention_mask[0, 0])
    if causal not in _cache:
        _cache[causal] = _build(causal)
    nc = _cache[causal]

    in_maps = _host_inputs(x, attention_mask, cos, sin, w_qkv, w_o, causal)
    try:
        res = run_bass_kernel_spmd(nc, in_maps, list(range(NCORES)), trace=_trace)
    except Exception:
        # transient device errors (e.g. NRT_EXEC_UNIT_UNRECOVERABLE) usually
        # clear on retry
        res = run_bass_kernel_spmd(nc, in_maps, list(range(NCORES)), trace=_trace)
    out = np.zeros((L, D), np.float64)
    for c in range(NCORES):
        out += res.results[c]["out_p"].astype(np.float64)
    if _trace:
        kernel._last_exec_time_ns = res.exec_time_ns
        kernel._last_res = res
    return out.astype(np.float32).reshape(B, L, D)


# revision 32
# speedup vs baseline: 1.0150x; 1.0090x over previous
"""Tensor-parallel attention kernel for Trainium2 (8 NeuronCores).

Problem: B=1, L=2048, D=4096, H=32 q-heads, KV=8 kv-heads, HD=128,
partial rotary ROT=64, causal additive mask, o-projection.

Sharding: TP-8 over heads. Core c owns q-heads 4c..4c+3 and kv-head c
(column shard of w_qkv), plus the matching row shard of w_o. Each core
computes a full [L, D] partial of the output; the host sums the 8
partials (the cross-core reduction of the row-sharded o-projection).

Precision plan: fp16 operands everywhere (PSUM accumulation fp32) —
~216 ns per N=512 matmul vs ~272 ns for fp32r, and half the HBM
traffic. (fp8 was simulated end-to-end and rejected: every placement
of e4m3 operands exceeds the 2e-2 rel-err budget — 2.5e-2..5.6e-2 —
and softmax probabilities overflow e4m3's +-240 range.) The exp is
shifted (exp(s-5)); the shift cancels in the normalization.

On-chip orientation: everything transposed so matmuls contract over
the partition dim with no activation transposes:
  qkvT[col, L] = w_qkv.T @ x.T          (w stationary, xT streamed)
  rope:  qT' = qT * cosE + shift32(qT) * sinE'
         (shift32 = swap of partition blocks 0:32/32:64 done by two
          SBUF->SBUF DMAs; the rotate-half sign is folded into sinE')
  ST[k, q]   = kT_tile.T @ qT            (one matmul per k-tile, K=HD=128)
  PT         = exp(ST - 5)  (diagonal tiles then get a 0/1 visibility mul)
  den[*, q]  = ones.T @ PT               (ones-matmul, accumulated over k)
  oT[d, q]   = V_tile.T @ PT             (V from a one-time PE transpose of vT)
  out[l, e]  = (oT/den).T @ w_o_shard    (partial; summed across cores on host)

Schedule (from trace analysis of the 413 us baseline):
  - lb0 qkv runs in two passes (ct 0-2, then ct 3-5) so the first
    block's weight-slab DMA demand is halved while x streams in; the
    3 DMA queues (sync/scalar HWDGE + gpsimd SWDGE) are balanced to
    ~<=1 MB per queue per cg.
  - PE warmup matmuls read a gpsimd-memset tile, so they start at
    ~6 us (vs waiting for a DMA) and the HAM clock-gate is at 8/8
    before the first real matmul.
  - attention and the o-projection are FUSED: o-proj chunks of q-block
    jq-1 are emitted between the attention heads of block jq, so the
    tensor engine always has ready matmuls to fill exp/normalize
    latency (den/ot accumulators are single-buffered to make the PSUM
    budget fit: ST 2x2 + den 1 + ot 1 + oproj 2 = 8 banks).
"""

import sys

for _p in ("/opt/trn_rl_repo", "/root/.axon_site/_ro/trn_rl_repo"):
    if _p not in sys.path:
        sys.path.append(_p)

import numpy as np

B, L, D = 1, 2048, 4096
H, KV, HD = 32, 8, 128
ROT = 64
SCALE = HD ** -0.5
NEG = -1e9
NCORES = 8
HPC = H // NCORES          # q-heads per core (4)
CPC = HPC * HD + 2 * HD    # w_qkv columns per core (768)
NDT = D // 128             # contraction tiles over D (32)
NKT = L // 128             # k tiles (16)
NJQ = L // 512             # 512-wide q blocks (4)
XBLK = 512                 # L-block width in the qkv phase
NLB = L // XBLK            # 4
EXPSHIFT = -5.0            # softmax exp shift; cancels in normalization
MASKNEG = -30000.0         # additive mask for diagonal tiles (fp16-safe)

_cache = {}


def _build(causal: bool):
    import concourse.mybir as mybir
    import concourse.tile as tile
    from concourse import bacc

    F32 = mybir.dt.float32
    F16 = mybir.dt.float16
    EXP = mybir.ActivationFunctionType.Exp

    nc = bacc.Bacc("TRN2", target_bir_lowering=False, debug=False)

    xt = nc.dram_tensor("xt", [NLB, 128, NDT, XBLK], F16, kind="ExternalInput").ap()
    # ct-major: [6, 128, NDT, 128] so lb0's pass A only gates on 3 slabs
    wqkv = nc.dram_tensor("wqkv", [6, 128, NDT, 128], F16, kind="ExternalInput").ap()
    # eg-major: [128, 2, HPC, 2048] so the first o-proj chunks only need
    # the first 2 MB half
    wo = nc.dram_tensor("wo", [128, 2, HPC, 2048], F16, kind="ExternalInput").ap()
    cos_e = nc.dram_tensor("cos_e", [NLB, 128, 2, XBLK], F16, kind="ExternalInput").ap()
    sin_e = nc.dram_tensor("sin_e", [NLB, 128, 2, XBLK], F16, kind="ExternalInput").ap()
    consts = nc.dram_tensor("consts", [128, 128], F16, kind="ExternalInput").ap()
    if causal:
        # 0/1 lower-triangle for the diagonal 128x128 blocks (the mask is
        # trivially 1 outside the block itself)
        dmadd = nc.dram_tensor("dmadd", [128, 128], F16,
                               kind="ExternalInput").ap()
    else:
        mask_t = nc.dram_tensor("mask_t", [L, L], F16, kind="ExternalInput").ap()
    out_p = nc.dram_tensor("out_p", [L, D], F16, kind="ExternalOutput").ap()

    with tile.TileContext(nc) as tc:
        with tc.tile_pool(name="persist", bufs=1) as persist:
            kt_sb = persist.tile([128, L], F16, tag="kt")
            v_sb = persist.tile([128, NKT, 128], F16, tag="v")
            qt_sb = persist.tile([128, HPC, L], F16, tag="qt")
            otn_sb = persist.tile([128, HPC, L], F16, tag="otn")
            ones = persist.tile([128, 128], F16, tag="ones")
            dm_sb = persist.tile([128, 128], F16, tag="dm")
            expb = persist.tile([128, 1], F32, tag="expb")
            warm = persist.tile([128, 384], F16, tag="warm")
            nc.gpsimd.memset(expb, EXPSHIFT)
            nc.gpsimd.memset(warm, 0.25)

            # ---------------- Phase 1: qkv projection + rope ----------------
            with tc.tile_pool(name="wq", bufs=1) as wqp, \
                 tc.tile_pool(name="xb", bufs=2) as xbp, \
                 tc.tile_pool(name="tabs", bufs=2) as tabs, \
                 tc.tile_pool(name="stage", bufs=3) as stage, \
                 tc.tile_pool(name="rotp", bufs=4) as rotp, \
                 tc.tile_pool(name="vtmp", bufs=2) as vtmp, \
                 tc.tile_pool(name="ps1", bufs=6, space="PSUM") as ps1, \
                 tc.tile_pool(name="psw", bufs=2, space="PSUM") as psw:
                wq_sb = wqp.tile([128, 6, NDT, 128], F16)

                # PE warm-up on a memset tile: starts ~6us in (no DMA dep)
                # so the HAM clock gate reaches 8/8 before the real matmuls
                for w_i in range(16):
                    wps = psw.tile([128, 384], F32, tag="warm",
                                   name=f"warm_{w_i}")
                    nc.tensor.matmul(out=wps, lhsT=warm[:, 0:128], rhs=warm,
                                     start=True, stop=True)

                # deferred tail-work (rope DVE / v transposes) per (lb, ct),
                # emitted one-to-two matmul-groups later so the PE never
                # stalls waiting on the ACT copy of a group's PSUM.
                pending = []

                def flush_pending(n=99):
                    while pending and n > 0:
                        pending.pop(0)()
                        n -= 1

                def post_group(lb, ct, acc, cosb, sinb):
                    # last block alternates drains across DVE/ACT so neither
                    # queue backlogs into the attention phase's start
                    last_lb = lb == NLB - 1
                    on_dve = last_lb and ct % 2 == 0
                    if ct == 5:
                        vt = vtmp.tile([128, XBLK], F16, tag="vt",
                                       name=f"vt_{lb}")
                        if on_dve:
                            nc.vector.tensor_copy(vt, acc)
                        else:
                            nc.scalar.copy(out=vt, in_=acc)

                        def fin_v(lb=lb, vt=vt):
                            # DMA-engine transpose: vT [128d, 512l] ->
                            # v [4x128 l-rows, 128 d], keeping the PE free
                            kk = XBLK // 128
                            nc.sync.dma_start_transpose(
                                out=v_sb[:, kk * lb:kk * (lb + 1), :], in_=vt)

                        pending.append(fin_v)
                        return
                    # rope for q (ct 0..3, scaled tables) and k (ct 4)
                    ti = 0 if ct < 4 else 1
                    s_sb = stage.tile([128, XBLK], F16, tag="s_sb",
                                      name=f"s_sb_{lb}_{ct}", bufs=6)
                    if on_dve:
                        nc.vector.tensor_copy(s_sb, acc)
                    else:
                        nc.scalar.copy(out=s_sb, in_=acc)
                    # rotate-half partition swap via SBUF->SBUF DMA, issued
                    # now so it lands before the deferred DVE work needs it
                    rot = rotp.tile([64, XBLK], F16, tag="rot",
                                    name=f"rot_{lb}_{ct}")
                    reng = (nc.sync, nc.scalar)[ct % 2]
                    reng.dma_start(out=rot[0:32, :], in_=s_sb[32:64, :])
                    reng.dma_start(out=rot[32:64, :], in_=s_sb[0:32, :])

                    def fin_rope(ct=ct, s_sb=s_sb, rot=rot, cosb=cosb,
                                 sinb=sinb, ti=ti, lb=lb):
                        ls = slice(lb * XBLK, (lb + 1) * XBLK)
                        dst = kt_sb[:, ls] if ct == 4 else qt_sb[:, ct, ls]
                        # last block's ropes split across GpSimd/DVE so the
                        # DVE queue is clear for jq0's softmax epilogue and
                        # GpSimd still reaches the w_o DMA issue promptly
                        eng = (nc.gpsimd if lb == NLB - 1 and ct % 2 == 0
                               else nc.vector)
                        eng.tensor_mul(dst, s_sb, cosb[:, ti, :])
                        m2 = stage.tile([64, XBLK], F16, tag="m2",
                                        name=f"m2_{lb}_{ct}")
                        eng.tensor_mul(m2, rot, sinb[0:64, ti, :])
                        eng.tensor_add(dst[0:64, :], dst[0:64, :], m2)

                    pending.append(fin_rope)

                for lb in range(NLB):
                    xblk = xbp.tile([128, NDT, XBLK], F16, tag="xblk")
                    cosb = tabs.tile([128, 2, XBLK], F16, tag="cosb")
                    sinb = tabs.tile([128, 2, XBLK], F16, tag="sinb")
                    if lb == 0:
                        # two-pass first block: pass A (ct 0-2) only needs
                        # half the weight slab while x streams in; queue
                        # plan per cg: gpsimd [x-half, ct2], scalar
                        # [x-half, ct5], sync [ct0, ct1]; ct3/ct4 follow
                        # on sync/gpsimd during pass A's compute.
                        for cg in range(4):
                            cgs = slice(cg * 8, cg * 8 + 8)
                            if cg == 0:
                                # quarter-granularity so the very first
                                # matmuls start as early as possible; sync
                                # is dedicated to w so ct0/ct1 land in step
                                # with the x quarters
                                nc.gpsimd.dma_start(out=xblk[:, 0:2, :],
                                                    in_=xt[lb, :, 0:2, :])
                                nc.scalar.dma_start(out=xblk[:, 2:4, :],
                                                    in_=xt[lb, :, 2:4, :])
                                nc.gpsimd.dma_start(out=xblk[:, 4:6, :],
                                                    in_=xt[lb, :, 4:6, :])
                                nc.scalar.dma_start(out=xblk[:, 6:8, :],
                                                    in_=xt[lb, :, 6:8, :])
                                nc.sync.dma_start(out=wq_sb[:, 0, 0:4, :],
                                                  in_=wqkv[0, :, 0:4, :])
                                nc.sync.dma_start(out=wq_sb[:, 0, 4:8, :],
                                                  in_=wqkv[0, :, 4:8, :])
                                nc.sync.dma_start(out=wq_sb[:, 1, 0:4, :],
                                                  in_=wqkv[1, :, 0:4, :])
                                nc.sync.dma_start(out=wq_sb[:, 1, 4:8, :],
                                                  in_=wqkv[1, :, 4:8, :])
                                nc.gpsimd.dma_start(out=wq_sb[:, 2, cgs, :],
                                                    in_=wqkv[2, :, cgs, :])
                            else:
                                nc.gpsimd.dma_start(
                                    out=xblk[:, cg * 8:cg * 8 + 4, :],
                                    in_=xt[lb, :, cg * 8:cg * 8 + 4, :])
                                nc.scalar.dma_start(
                                    out=xblk[:, cg * 8 + 4:cg * 8 + 8, :],
                                    in_=xt[lb, :, cg * 8 + 4:cg * 8 + 8, :])
                                nc.sync.dma_start(out=wq_sb[:, 0, cgs, :],
                                                  in_=wqkv[0, :, cgs, :])
                                nc.sync.dma_start(out=wq_sb[:, 1, cgs, :],
                                                  in_=wqkv[1, :, cgs, :])
                                nc.gpsimd.dma_start(out=wq_sb[:, 2, cgs, :],
                                                    in_=wqkv[2, :, cgs, :])
                        # pass-B slabs + ct5 behind all of pass A's traffic
                        for cg in range(4):
                            cgs = slice(cg * 8, cg * 8 + 8)
                            nc.sync.dma_start(out=wq_sb[:, 3, cgs, :],
                                              in_=wqkv[3, :, cgs, :])
                            nc.gpsimd.dma_start(out=wq_sb[:, 4, cgs, :],
                                                in_=wqkv[4, :, cgs, :])
                            nc.scalar.dma_start(out=wq_sb[:, 5, cgs, :],
                                                in_=wqkv[5, :, cgs, :])
                        # rope tables aren't read until the first rope finish
                        # (~25us in); keep them behind the critical x chunks
                        nc.scalar.dma_start(out=cosb, in_=cos_e[lb])
                        nc.scalar.dma_start(out=sinb, in_=sin_e[lb])
                        accs0 = {ct: ps1.tile([128, XBLK], F32, tag="acc",
                                              name=f"acc0_{ct}")
                                 for ct in range(6)}
                        for cts in ((0, 1, 2), (3, 4, 5)):
                            for cg in range(4):
                                for ct in cts:
                                    for dti in range(cg * 8, cg * 8 + 8):
                                        nc.tensor.matmul(
                                            out=accs0[ct],
                                            lhsT=wq_sb[:, ct, dti, :],
                                            rhs=xblk[:, dti, :],
                                            start=(dti == 0),
                                            stop=(dti == NDT - 1))
                            for ct in cts:
                                post_group(lb, ct, accs0[ct], cosb, sinb)
                        continue
                    # chunked so block-1 matmuls can start before the whole
                    # 4 MB block has landed (blocks 2-3 are prefetched anyway);
                    # block 1 splits across two queues since it races block-0
                    # traffic
                    for cg in range(4):
                        xeng = nc.scalar if (lb == 1 and cg % 2 == 1) else nc.gpsimd
                        xeng.dma_start(out=xblk[:, cg * 8:cg * 8 + 8, :],
                                       in_=xt[lb, :, cg * 8:cg * 8 + 8, :])
                    nc.sync.dma_start(out=cosb, in_=cos_e[lb])
                    nc.sync.dma_start(out=sinb, in_=sin_e[lb])
                    if lb == 1 and causal:
                        # needed from phase 2 on; off the hot queues
                        nc.scalar.dma_start(out=dm_sb, in_=dmadd)
                    for ct in range(6):
                        acc = ps1.tile([128, XBLK], F32, tag="acc",
                                       name=f"acc_{lb}_{ct}")
                        for dti in range(NDT):
                            nc.tensor.matmul(
                                out=acc,
                                lhsT=wq_sb[:, ct, dti, :],
                                rhs=xblk[:, dti, :],
                                start=(dti == 0), stop=(dti == NDT - 1))
                        # drain faster in the last block so the rope tail
                        # doesn't delay the phase-2 PSUM handoff
                        flush_pending(3 if lb == NLB - 1 else 2)
                        post_group(lb, ct, acc, cosb, sinb)
                flush_pending()
                # ones for the den matmuls: first read at jq0, tiny transfer
                nc.sync.dma_start(out=ones, in_=consts)

            # ---------------- Fused phase 2+3: attention + o-proj -----------
            # PSUM budget: ST 2x[128,2,512] (4 banks) + den (1) + ot (1)
            # + o-proj accs 2x (2) = 8 banks.
            with tc.tile_pool(name="wop", bufs=1) as wop, \
                 tc.tile_pool(name="ptp", bufs=8) as ptp, \
                 tc.tile_pool(name="mb", bufs=2) as mbp, \
                 tc.tile_pool(name="rdp", bufs=3) as rdp, \
                 tc.tile_pool(name="ostp", bufs=4) as ostp, \
                 tc.tile_pool(name="ps_st", bufs=2, space="PSUM") as ps_st, \
                 tc.tile_pool(name="ps_den", bufs=1, space="PSUM") as ps_den, \
                 tc.tile_pool(name="ps_ot", bufs=1, space="PSUM") as ps_ot, \
                 tc.tile_pool(name="ps3", bufs=2, space="PSUM") as ps3:
                wo_sb = wop.tile([128, 2, HPC, 2048], F16)
                nc.gpsimd.dma_start(out=wo_sb[:, 0], in_=wo[:, 0])
                nc.sync.dma_start(out=wo_sb[:, 1], in_=wo[:, 1])

                # o-proj chunk queue: chunks for q-block jq become ready when
                # jq's last head normalizes; they're emitted between the
                # attention heads of jq+1 as PE filler work.
                chunks = []          # pending (lt, eg) descriptors

                def emit_chunk(lt, eg, tail=False):
                    lsl = slice(lt * 128, (lt + 1) * 128)
                    ost = ostp.tile([128, 4, 512], F16, tag="ost",
                                    name=f"ost_{lt}_{eg}")
                    for e4 in range(4):
                        acc = ps3.tile([128, 512], F32, tag="acc3",
                                       name=f"acc3_{lt}_{eg}_{e4}")
                        for h in range(HPC):
                            nc.tensor.matmul(
                                out=acc,
                                lhsT=otn_sb[:, h, lsl],
                                rhs=wo_sb[:, eg, h, e4 * 512:(e4 + 1) * 512],
                                start=(h == 0), stop=(h == HPC - 1))
                        # drains on DVE: the ACT queue stays clear for exps.
                        # The kernel-tail chunks split across both engines
                        # and DMA out in halves so the drain isn't serial.
                        if tail and e4 % 2 == 1:
                            nc.scalar.copy(out=ost[:, e4, :], in_=acc)
                        else:
                            nc.vector.tensor_copy(ost[:, e4, :], acc)
                        if tail and e4 == 1:
                            nc.sync.dma_start(
                                out=out_p[lsl, eg * 2048:eg * 2048 + 1024],
                                in_=ost[:, 0:2, :])
                    if tail:
                        nc.scalar.dma_start(
                            out=out_p[lsl, eg * 2048 + 1024:(eg + 1) * 2048],
                            in_=ost[:, 2:4, :])
                    else:
                        nc.sync.dma_start(
                            out=out_p[lsl, eg * 2048:(eg + 1) * 2048], in_=ost)

                def drain_chunks(n, final=False):
                    while chunks and n > 0:
                        emit_chunk(*chunks.pop(0), tail=final and len(chunks) < 2)
                        n -= 1

                for jq in range(NJQ):
                    qs = slice(jq * 512, (jq + 1) * 512)
                    nkt = 4 * (jq + 1) if causal else NKT
                    diag0 = 4 * jq
                    if not causal:
                        mblk = mbp.tile([128, NKT, 512], F16, tag="mblk")
                        nc.sync.dma_start(
                            out=mblk,
                            in_=mask_t[:, qs].rearrange("(kt p) q -> p kt q", p=128))
                    npair = nkt // 2
                    # diagonal pairs first: their DVE mask-muls then
                    # overlap later pairs' matmuls instead of sitting on
                    # the exp critical path at the iteration tail
                    if causal and npair >= 2:
                        order = [npair - 2, npair - 1] + list(range(npair - 2))
                    else:
                        order = list(range(npair))

                    hctx = {}

                    def get_ctx(h, jq=jq):
                        if h not in hctx:
                            # jq0 has no o-proj filler yet, so its odd heads
                            # borrow the (idle) o-proj acc banks to avoid
                            # den/ot bank serialization at head boundaries
                            if jq == 0 and h % 2 == 1:
                                pd = po = ps3
                                td = to = "acc3"
                            else:
                                pd, po, td, to = ps_den, ps_ot, "den", "ot"
                            hctx[h] = {
                                "den": pd.tile([128, 512], F32, tag=td,
                                               name=f"den_{jq}_{h}"),
                                "ot": po.tile([128, 512], F32, tag=to,
                                              name=f"ot_{jq}_{h}"),
                                "n": 0,
                            }
                        return hctx[h]

                    def tile_off(i, diag0=diag0):
                        # visible region of k-tile i is q >= 128*o for
                        # diagonal offset o; off-diagonal tiles are full
                        if not causal or i < diag0:
                            return 0
                        return 128 * (i - diag0)

                    def emit_den_ot(h, g, pts, jq=jq, diag0=diag0,
                                    npair=npair, tile_off=tile_off,
                                    get_ctx=get_ctx):
                        c = get_ctx(h)
                        pt = pts.pop((h, g))
                        if causal:
                            # 0/1 visibility applied to the fp16
                            # probabilities, off the exp critical path;
                            # only the diagonal 128x128 block is nontrivial
                            for t in (0, 1):
                                i = 2 * g + t
                                if i >= diag0:
                                    q0 = 128 * (i - diag0)
                                    nc.vector.tensor_mul(
                                        pt[:, t, q0:q0 + 128],
                                        pt[:, t, q0:q0 + 128], dm_sb)
                        for t in (0, 1):
                            i = 2 * g + t
                            q0 = tile_off(i)
                            first = c["n"] == 0
                            last = c["n"] == 2 * npair - 1
                            # the first emitted matmul is always the
                            # full-width diagonal o=0 tile, so start=True
                            # initializes the whole PSUM row
                            nc.tensor.matmul(
                                out=c["den"][:, q0:], lhsT=ones,
                                rhs=pt[:, t, q0:],
                                start=first, stop=last)
                            nc.tensor.matmul(
                                out=c["ot"][:, q0:], lhsT=v_sb[:, i, :],
                                rhs=pt[:, t, q0:],
                                start=first, stop=last)
                            c["n"] += 1
                        if last:
                            rd = rdp.tile([128, 512], F32, tag="rd",
                                          name=f"rd_{jq}_{h}")
                            nc.vector.reciprocal_approx_fast(
                                out=rd, in_=c["den"])
                            nc.vector.tensor_mul(
                                otn_sb[:, h, qs], c["ot"], rd)
                            # PE filler: o-proj chunks of the previous block
                            drain_chunks(2)

                    pts = {}
                    inflight = []
                    la = 4 if jq == 0 else 2
                    for h in range(HPC):
                        for g in order:
                            st = ps_st.tile([128, 2, 512], F32, tag="st",
                                            name=f"st_{jq}_{h}_{g}")
                            for t in (0, 1):
                                i = 2 * g + t
                                q0 = tile_off(i)
                                nc.tensor.matmul(
                                    out=st[:, t, q0:],
                                    lhsT=kt_sb[:, i * 128:(i + 1) * 128],
                                    rhs=qt_sb[:, h, jq * 512 + q0:(jq + 1) * 512],
                                    start=True, stop=True)
                            if not causal:
                                for t in (0, 1):
                                    nc.vector.tensor_add(
                                        st[:, t, :], st[:, t, :],
                                        mblk[:, 2 * g + t, :])
                            pt = ptp.tile([128, 2, 512], F16, tag="pt",
                                          name=f"pt_{jq}_{h}_{g}")
                            if causal and 2 * g + 1 >= diag0:
                                # diagonal pairs: exp per k-tile halves the
                                # ST->exp->den/ot latency where the narrow
                                # matmuls can't hide a full-pair exp
                                for t in (0, 1):
                                    nc.scalar.activation(
                                        pt[:, t, :], st[:, t, :], EXP,
                                        bias=expb)
                            else:
                                nc.scalar.activation(pt, st, EXP, bias=expb)
                            pts[(h, g)] = pt
                            inflight.append((h, g))
                            if len(inflight) > la:
                                emit_den_ot(*inflight.pop(0), pts)
                    while inflight:
                        emit_den_ot(*inflight.pop(0), pts)
                    # queue this block's o-proj chunks (eg-major so the
                    # first chunks only need the first half of w_o)
                    for eg in range(2):
                        for lt in range(4 * jq, 4 * jq + 4):
                            chunks.append((lt, eg))
                # tail: the last block's chunks
                drain_chunks(99, final=True)

    nc.compile()
    return nc


def _host_inputs(x, attention_mask, cos, sin, w_qkv, w_o, causal):
    """Build the 8 per-core input maps (fp16, partition-major)."""
    F16 = np.float16

    # x.T pre-tiled: [NLB, 128, NDT, XBLK], contiguous per partition
    xT = np.ascontiguousarray(x[0].T)                     # [D, L]
    xt_h = np.ascontiguousarray(
        xT.reshape(NDT, 128, NLB, XBLK).transpose(2, 1, 0, 3)).astype(F16)
    q_pos = H * HD
    kv_pos = q_pos + KV * HD

    # extended rope tables: slot 0 = q (scale folded), slot 1 = k
    # row d<64: cos[l, d]; row d>=64: 1.0 (cos) / 0.0 (sin)
    # rotate-half sign folded into sin: rows 0:32 negated
    cos_t = cos.T.astype(np.float32)                      # [ROT, L]
    sin_t = sin.T.astype(np.float32)
    cos_e = np.empty((2, 128, L), np.float32)
    sin_e = np.zeros((2, 128, L), np.float32)
    cos_e[0, :ROT] = cos_t * SCALE
    cos_e[0, ROT:] = SCALE
    cos_e[1, :ROT] = cos_t
    cos_e[1, ROT:] = 1.0
    sin_e[0, :ROT] = sin_t * SCALE
    sin_e[1, :ROT] = sin_t
    sin_e[:, :32] *= -1.0
    # -> [NLB, 128, 2, XBLK]
    cs_h = np.ascontiguousarray(
        cos_e.reshape(2, 128, NLB, XBLK).transpose(2, 1, 0, 3)).astype(F16)
    sn_h = np.ascontiguousarray(
        sin_e.reshape(2, 128, NLB, XBLK).transpose(2, 1, 0, 3)).astype(F16)

    consts = np.ones((128, 128), np.float32).astype(F16)

    mask2d = np.ascontiguousarray(attention_mask[0, 0])   # [L(q), L(k)]
    if causal:
        mask_t_full = None
        # 0/1 lower-triangle [k_local, q_local] for the diagonal blocks
        kloc = np.arange(128)[:, None]
        qloc = np.arange(128)[None, :]
        dmadd = np.ascontiguousarray((qloc >= kloc).astype(F16))
    else:
        mask_t_full = np.ascontiguousarray(
            np.maximum(mask2d.T, MASKNEG)).astype(F16)    # [k, q]
        dmadd = None

    in_maps = []
    for c in range(NCORES):
        cols = []
        for j in range(HPC):
            h = c * HPC + j
            cols.append(w_qkv[:, h * HD:(h + 1) * HD])
        cols.append(w_qkv[:, q_pos + c * HD:q_pos + (c + 1) * HD])
        cols.append(w_qkv[:, kv_pos + c * HD:kv_pos + (c + 1) * HD])
        wqkv_c = np.concatenate(cols, axis=1)             # [D, 768]
        # ct-major slabs: [6, 128, NDT, 128]
        wqkv_h = np.ascontiguousarray(
            wqkv_c.reshape(NDT, 128, 6, 128).transpose(2, 1, 0, 3)).astype(F16)
        wo_c = w_o[c * HPC * HD:(c + 1) * HPC * HD, :]    # [512, D]
        # eg-major: [128, 2, HPC, 2048]
        wo_h = np.ascontiguousarray(
            wo_c.reshape(HPC, 128, 2, 2048).transpose(1, 2, 0, 3)).astype(F16)
        m = {"xt": xt_h, "wqkv": wqkv_h, "wo": wo_h,
             "cos_e": cs_h, "sin_e": sn_h, "consts": consts}
        if causal:
            m["dmadd"] = dmadd
        else:
            m["mask_t"] = mask_t_full
        in_maps.append(m)
    return in_maps


def _is_causal(mask2d):
    expected = np.where(
        np.tril(np.ones((L, L), dtype=bool)), np.float32(0.0), np.float32(NEG))
    return mask2d.shape == (L, L) and np.array_equal(mask2d, expected)


def kernel(x, attention_mask, cos, sin, w_qkv, w_o, _trace=False):
    from concourse.bass_utils import run_bass_kernel_spmd

    x = np.asarray(x, dtype=np.float32)
    attention_mask = np.asarray(attention_mask, dtype=np.float32)
    cos = np.asarray(cos, dtype=np.float32)
    sin = np.asarray(sin, dtype=np.float32)
    w_qkv = np.asarray(w_qkv, dtype=np.float32)
    w_o = np.asarray(w_o, dtype=np.float32)

    causal = _is_causal(attention_mask[0, 0])
    if causal not in _cache:
        _cache[causal] = _build(causal)
    nc = _cache[causal]

    in_maps = _host_inputs(x, attention_mask, cos, sin, w_qkv, w_o, causal)
    try:
        res = run_bass_kernel_spmd(nc, in_maps, list(range(NCORES)), trace=_trace)
    except Exception:
        # transient device errors (e.g. NRT_EXEC_UNIT_UNRECOVERABLE) usually
        # clear on retry
        res = run_bass_kernel_spmd(nc, in_maps, list(range(NCORES)), trace=_trace)
    out = np.zeros((L, D), np.float64)
    for c in range(NCORES):
        out += res.results[c]["out_p"].astype(np.float64)
    if _trace:
        kernel._last_exec_time_ns = res.exec_time_ns
        kernel._last_res = res
    return out.astype(np.float32).reshape(B, L, D)
